# revision 1
# baseline (speedup 1.0000x reference)
"""Trainium2 Bass kernel for FlowNet-style CorrelationLayer.

out[b, di*21+dj, h, w] = (1/C) * sum_c feat1[b,c,h,w] * feat2p[b,c,h+di,w+dj]
with feat1/feat2 [4, 64, 128, 128] f32, MAX_DISP=10 (21x21=441 displacements).

Sharding: 8 cores = (batch b in 0..3) x (H-half hh in 0..1). Each core gets
feat1 rows [64] (pre-scaled by 1/C) and a zero-padded feat2 slab with 10-row
halo and 10-col pads.

Per core, for each local output row h and di-batch, TensorE computes the
"gram band" G[w, di, w2] = sum_c f1[c,h,w] * f2p[c,h+di,w2] restricted to the
52-wide window w2 in [32*(w//32), 32*(w//32)+52) via four column-tiled
matmuls (tile_position col groups, float32r at full rate, contraction over
C=64 partitions). Vector+Scalar engines drain PSUM->SBUF (casting to the
stage dtype), and one contiguous DMA per row ships [128, 21, 52] to DRAM.
The host unshard step gathers the diagonal band (w2 = w%32 + dj), which no
on-chip engine can address (per-partition-varying offsets are not
expressible in hardware access patterns).
"""

import sys

sys.path.insert(0, "/opt/trn_rl_repo")

import numpy as np
import ml_dtypes

import concourse.bass as bass
import concourse.tile as tile
from concourse import bacc, mybir
import concourse.bass_utils as bass_utils

B, C, H, W = 4, 64, 128, 128
MD = 10
KS = 2 * MD + 1  # 21
D = KS * KS  # 441
HL = H // 2  # 64 local rows per core
HP = HL + 2 * MD  # 84 padded feat2 rows
WP = W + 2 * MD  # 148 padded feat2 cols
GW = 32 + KS - 1  # 52: trimmed window width per 32-col group
NG = 4  # column groups
# di batches; each batch's matmul moving size nb*GW must be >= 256 for
# float32r full rate (1 cycle/row).
DI_BATCHES = [(0, 6), (6, 5), (11, 5), (16, 5)]

F32 = mybir.dt.float32
F32R = mybir.dt.float32r
BF16 = mybir.dt.bfloat16

STAGE_DT = BF16  # bf16 halves output DMA; host converts back to f32

_NC_CACHE = {}


def _build_nc():
    nc = bacc.Bacc("TRN2", target_bir_lowering=False, debug=False, num_devices=8)
    f1 = nc.dram_tensor("f1", [C, HL, W], BF16, kind="ExternalInput").ap()
    f2 = nc.dram_tensor("f2", [C, HP, WP], BF16, kind="ExternalInput").ap()
    scr = nc.dram_tensor("scr", [HL, W, KS, GW], STAGE_DT, kind="ExternalOutput").ap()

    with tile.TileContext(nc) as tc:
        with (
            tc.tile_pool(name="inp", bufs=1) as inp,
            tc.tile_pool(name="ps", bufs=8, space=bass.MemorySpace.PSUM) as psp,
            tc.tile_pool(name="st", bufs=4) as stp,
        ):
            t1 = inp.tile([C, HL, W], BF16)
            t2 = inp.tile([C, HP, WP], BF16)
            nc.sync.dma_start(t1[:], f1[:])
            nc.sync.dma_start(t2[:], f2[:])

            for h in range(HL):
                stage = stp.tile([128, KS, GW], STAGE_DT)
                for bi, (d0, nb) in enumerate(DI_BATCHES):
                    pt = psp.tile([128, nb, GW], F32, tag="pt")
                    for g in range(NG):
                        nc.tensor.matmul(
                            pt[32 * g : 32 * g + 32, :, :],
                            t1[:, h, 32 * g : 32 * g + 32],
                            t2[:, h + d0 : h + d0 + nb, 32 * g : 32 * g + GW],
                            start=True,
                            stop=True,
                            tile_position=(0, 32 * g),
                        )
                    dst = stage[:, d0 : d0 + nb, :]
                    if bi % 2 == 0:
                        nc.vector.tensor_copy(dst, pt[:])
                    else:
                        nc.scalar.copy(dst, pt[:])
                nc.sync.dma_start(scr[h], stage[:])
    nc.compile()
    return nc


def _get_nc():
    if "nc" not in _NC_CACHE:
        _NC_CACHE["nc"] = _build_nc()
    return _NC_CACHE["nc"]


def _shard_inputs(feat1, feat2):
    f1 = np.asarray(feat1, dtype=np.float32)
    f2 = np.asarray(feat2, dtype=np.float32)
    in_maps = []
    for k in range(8):
        b, hh = k // 2, k % 2
        h0 = hh * HL
        f1c = np.ascontiguousarray(f1[b, :, h0 : h0 + HL, :]) * np.float32(1.0 / C)
        f2pc = np.zeros((C, HP, WP), dtype=np.float32)
        r0 = h0 - MD
        s0, s1 = max(r0, 0), min(h0 + HL + MD, H)
        f2pc[:, s0 - r0 : s1 - r0, MD : MD + W] = f2[b, :, s0:s1, :]
        in_maps.append(
            {"f1": f1c.astype(ml_dtypes.bfloat16), "f2": f2pc.astype(ml_dtypes.bfloat16)}
        )
    return in_maps


# idx[w, dj] = (w % 32) + dj, the window-local column of band element dj
_GATHER_IDX = ((np.arange(W) % 32)[:, None] + np.arange(KS)[None, :])[None, None]


def _assemble(results):
    out = np.empty((B, D, H, W), dtype=np.float32)
    for k in range(8):
        b, hh = k // 2, k % 2
        blk = np.asarray(results[k]["scr"], dtype=np.float32)  # [h, w, di, GW]
        arr = blk.transpose(0, 2, 1, 3)  # [h, di, w, GW]
        gat = np.take_along_axis(arr, _GATHER_IDX, axis=-1)  # [h, di, w, dj]
        oc = gat.transpose(1, 3, 0, 2).reshape(D, HL, W)  # [di*21+dj, h, w]
        out[b, :, hh * HL : (hh + 1) * HL, :] = oc
    return out


def run(feat1, feat2, **spmd_kwargs):
    nc = _get_nc()
    in_maps = _shard_inputs(feat1, feat2)
    res = bass_utils.run_bass_kernel_spmd(
        nc, in_maps, core_ids=list(range(8)), **spmd_kwargs
    )
    return _assemble(res.results), res


def kernel(feat1, feat2):
    out, _ = run(feat1, feat2)
    return out

